# revision 11
# baseline (speedup 1.0000x reference)
"""GRU memory updater (scatter_memory) on 8 Trainium2 NeuronCores.

Strategy (node partitioning, per sharding hint):
  - The 1M x 172 node-memory table is sharded row-wise: core k owns rows
    [k*125000, (k+1)*125000).
  - Host routes (id, message, ts) triples to the owning core, sorts by local
    row id, pads to a common capacity, and pre-transposes messages (X^T) so
    the device never has to transpose the message operand.
  - Per core the bass kernel:
      * bulk-copies its memory shard DRAM->DRAM into the output (the
        memory-roofline term),
      * gathers the ~25k updated rows via indirect DMA, runs the GRU cell
        (PE matmuls with fp32, gates fused into PSUM accumulation, biases
        folded in via ones-rows), and
      * scatters updated rows + timestamps into the output shard.
  - Host strips padding and concatenates shards back to the full output.

kernel(**inputs) takes the FULL unsharded inputs and returns the full
(updated_memory[:, None, :], updated_last_update) tuple, matching the
reference.
"""

import contextlib
import ctypes
import math
import os
import sys
import types

# Environment bootstrap: concourse/trn deps resolve via the axon site dirs.
for _p in (
    "/root/.axon_site",
    "/root/.axon_site/_ro/trn_rl_repo",
    "/root/.axon_site/_ro/pypackages",
    "/opt/trn_rl_repo",
):
    if os.path.isdir(_p) and _p not in sys.path:
        sys.path.append(_p)

import numpy as np

import concourse.bacc as bacc
import concourse.bass as bass
import concourse.mybir as mybir
import concourse.tile as tile
from concourse.bass_utils import run_bass_kernel_spmd
from concourse.masks import make_identity

AFT = mybir.ActivationFunctionType
F32 = mybir.dt.float32
I32 = mybir.dt.int32

N_NODES = 1_000_000
N_CORES = 8
D = 172  # memory/message feature dim
H3 = 3 * D  # 516 gate columns (r | z | n)
RZ = 2 * D  # 344 (r|z block)
R_SHARD = N_NODES // N_CORES  # 125000 rows per core

G = 16  # row-tiles (of 128 rows) per gather/scatter group
NCHUNK = 16  # bulk-copy chunks


def build_program(r_shard: int, nt: int, g: int, nchunk: int):
    """Build + compile the per-core bass program.

    r_shard: owned rows per core (shard tensors get one extra trash row).
    nt: number of 128-row update tiles (padded capacity = nt*128 rows).
    """
    ng = nt // g
    assert ng * g == nt
    mcap = nt * 128
    rows = r_shard + 1  # + trash row for padded scatter entries

    nc = bacc.Bacc(
        "TRN2", target_bir_lowering=False, debug=False, num_devices=N_CORES
    )

    mem_in = nc.dram_tensor("mem_in", (rows, D), F32, kind="ExternalInput").ap()
    lu_in = nc.dram_tensor("lu_in", (rows,), F32, kind="ExternalInput").ap()
    xt1_d = nc.dram_tensor("xt1", (128, mcap), F32, kind="ExternalInput").ap()
    xt2_d = nc.dram_tensor("xt2", (45, mcap), F32, kind="ExternalInput").ap()
    ids_d = nc.dram_tensor("ids", (128, nt), I32, kind="ExternalInput").ap()
    wih1_d = nc.dram_tensor("wih1", (128, H3), F32, kind="ExternalInput").ap()
    wih2_d = nc.dram_tensor("wih2", (45, H3), F32, kind="ExternalInput").ap()
    whh1_d = nc.dram_tensor("whh1", (128, H3), F32, kind="ExternalInput").ap()
    whh2_d = nc.dram_tensor("whh2", (45, H3), F32, kind="ExternalInput").ap()
    mem_out = nc.dram_tensor("mem_out", (rows, D), F32, kind="ExternalOutput").ap()
    lu_out = nc.dram_tensor("lu_out", (rows,), F32, kind="ExternalOutput").ap()

    with tile.TileContext(nc) as tc:
        with (
            tc.tile_pool(name="const", bufs=1) as const,
            tc.tile_pool(name="sb", bufs=2) as sb,
            tc.tile_pool(name="psum", bufs=2, space="PSUM") as ps,
        ):
            # Bulk shard copy, first in program order so its DMAs lead.
            step = math.ceil(rows / nchunk)
            for i in range(nchunk):
                a = i * step
                b = min(rows, a + step)
                if a >= b:
                    break
                nc.sync.dma_start(out=mem_out[a:b, :], in_=mem_in[a:b, :])
            nc.sync.dma_start(out=lu_out[None, :], in_=lu_in[None, :])

            # Constants: identity (for PE transpose), weights, ids, ts.
            ident = const.tile([128, 128], F32)
            make_identity(nc, ident[:])
            wih1 = const.tile([128, H3], F32)
            nc.scalar.dma_start(out=wih1[:], in_=wih1_d[:, :])
            wih2 = const.tile([45, H3], F32)
            nc.scalar.dma_start(out=wih2[:], in_=wih2_d[:, :])
            whh1 = const.tile([128, H3], F32)
            nc.scalar.dma_start(out=whh1[:], in_=whh1_d[:, :])
            whh2 = const.tile([45, H3], F32)
            nc.scalar.dma_start(out=whh2[:], in_=whh2_d[:, :])
            ids_sb = const.tile([128, nt], I32)
            nc.scalar.dma_start(out=ids_sb[:], in_=ids_d[:, :])

            for grp in range(ng):
                cs = grp * g * 128  # column start within mcap
                csl = slice(cs, cs + g * 128)

                xt1g = sb.tile([128, g * 128], F32, tag="xt1g")
                nc.scalar.dma_start(out=xt1g[:], in_=xt1_d[:, csl])
                xt2g = sb.tile([45, g * 128], F32, tag="xt2g")
                nc.scalar.dma_start(out=xt2g[:], in_=xt2_d[:, csl])

                for j in range(g):
                    jt = grp * g + j  # global tile index
                    idj = ids_sb[:, jt:jt + 1]
                    xs = slice(j * 128, (j + 1) * 128)

                    hjt = sb.tile([128, D], F32, tag="hj")
                    nc.gpsimd.indirect_dma_start(
                        out=hjt[:],
                        out_offset=None,
                        in_=mem_in,
                        in_offset=bass.IndirectOffsetOnAxis(ap=idj, axis=0),
                    )
                    hj = hjt[:]

                    # H^T via PE transpose (172 = 128 + 44 feature chunks).
                    tr1 = ps.tile([128, 128], F32, tag="tr")
                    nc.tensor.transpose(out=tr1[:], in_=hj[:, 0:128], identity=ident[:])
                    tr2 = ps.tile([128, 128], F32, tag="tr")
                    nc.tensor.transpose(
                        out=tr2[:44, :], in_=hj[:, 128:172], identity=ident[:]
                    )
                    ht1 = sb.tile([128, 128], F32, tag="ht1")
                    nc.vector.tensor_copy(out=ht1[:], in_=tr1[:])
                    # ht2[0:44] = H^T feats 128:172, ht2[44] = ones (bias row).
                    # memset covers [32:64) first; the copy then overwrites
                    # [0:44) — engines need 32-aligned partition starts.
                    ht2 = sb.tile([64, 128], F32, tag="ht2")
                    nc.vector.memset(ht2[32:64, :], 1.0)
                    nc.vector.tensor_copy(out=ht2[0:44, :], in_=tr2[:44, :])

                    # Gate pre-activations; gi+gh fused via PSUM accumulation.
                    rz_ps = ps.tile([128, RZ], F32, tag="rz")
                    nc.tensor.matmul(out=rz_ps[:], lhsT=xt1g[:, xs], rhs=wih1[:, 0:RZ], start=True, stop=False)
                    nc.tensor.matmul(out=rz_ps[:], lhsT=xt2g[:, xs], rhs=wih2[:, 0:RZ], start=False, stop=False)
                    nc.tensor.matmul(out=rz_ps[:], lhsT=ht1[:], rhs=whh1[:, 0:RZ], start=False, stop=False)
                    nc.tensor.matmul(out=rz_ps[:], lhsT=ht2[0:45, :], rhs=whh2[:, 0:RZ], start=False, stop=True)
                    in_ps = ps.tile([128, D], F32, tag="inp")
                    nc.tensor.matmul(out=in_ps[:], lhsT=xt1g[:, xs], rhs=wih1[:, RZ:H3], start=True, stop=False)
                    nc.tensor.matmul(out=in_ps[:], lhsT=xt2g[:, xs], rhs=wih2[:, RZ:H3], start=False, stop=True)
                    hn_ps = ps.tile([128, D], F32, tag="hn")
                    nc.tensor.matmul(out=hn_ps[:], lhsT=ht1[:], rhs=whh1[:, RZ:H3], start=True, stop=False)
                    nc.tensor.matmul(out=hn_ps[:], lhsT=ht2[0:45, :], rhs=whh2[:, RZ:H3], start=False, stop=True)

                    # r|z = sigmoid(rz), n = tanh(i_n + r*h_n), h' = n + z*(h-n)
                    rzs = sb.tile([128, RZ], F32, tag="rzs")
                    nc.scalar.activation(out=rzs[:], in_=rz_ps[:], func=AFT.Sigmoid)
                    t = sb.tile([128, D], F32, tag="t")
                    nc.vector.tensor_mul(out=t[:], in0=rzs[:, 0:D], in1=hn_ps[:])
                    nc.vector.tensor_add(out=t[:], in0=t[:], in1=in_ps[:])
                    nsb = sb.tile([128, D], F32, tag="nsb")
                    nc.scalar.activation(out=nsb[:], in_=t[:], func=AFT.Tanh)
                    dsb = sb.tile([128, D], F32, tag="dsb")
                    nc.vector.tensor_sub(out=dsb[:], in0=hj, in1=nsb[:])
                    nc.vector.tensor_mul(out=dsb[:], in0=dsb[:], in1=rzs[:, D:RZ])
                    hnw = sb.tile([128, D], F32, tag="hnw")
                    nc.vector.tensor_add(out=hnw[:], in0=nsb[:], in1=dsb[:])

                    nc.gpsimd.indirect_dma_start(
                        out=mem_out,
                        out_offset=bass.IndirectOffsetOnAxis(ap=idj, axis=0),
                        in_=hnw[:],
                        in_offset=None,
                    )

    nc.compile()
    return nc


def prepare_core_inputs(memory, last_update, ids64, msgs, ts, r_shard, mcap, core):
    """Route + pad this core's update stream and shard tensors."""
    lo = core * r_shard
    sel = np.nonzero((ids64 >= lo) & (ids64 < lo + r_shard))[0]
    loc = (ids64[sel] - lo).astype(np.int32)
    order = np.argsort(loc)
    sel = sel[order]
    loc = loc[order]
    mk = len(sel)
    assert mk <= mcap

    nt = mcap // 128
    ids_pad = np.full(mcap, r_shard, np.int32)  # pads hit the trash row
    ids_pad[:mk] = loc

    xt = np.zeros((173, mcap), np.float32)
    xt[0:172, :mk] = msgs[sel].T
    xt[172, :] = 1.0  # ones row -> folded biases

    mem_shard = np.concatenate(
        [memory[lo:lo + r_shard], np.zeros((1, D), np.float32)], axis=0
    )
    # last_update's new values are pure inputs (timestamps): pre-scatter on
    # host; the device moves the bytes (shard copy) like any other row.
    lu_shard = np.empty(r_shard + 1, np.float32)
    lu_shard[:r_shard] = last_update[lo:lo + r_shard]
    lu_shard[loc] = ts[sel]
    lu_shard[r_shard] = 0.0

    return {
        "mem_in": np.ascontiguousarray(mem_shard),
        "lu_in": np.ascontiguousarray(lu_shard),
        "xt1": np.ascontiguousarray(xt[0:128]),
        "xt2": np.ascontiguousarray(xt[128:173]),
        "ids": np.ascontiguousarray(ids_pad.reshape(nt, 128).T),
    }


def make_weight_inputs(weight_ih, weight_hh, bias_ih, bias_hh):
    wihT = weight_ih.T.astype(np.float32)  # [172, 516]
    whhT = weight_hh.T.astype(np.float32)
    bias_row_ih = np.empty((1, H3), np.float32)
    bias_row_ih[0, 0:RZ] = (bias_ih + bias_hh)[0:RZ]  # r|z biases, both halves
    bias_row_ih[0, RZ:H3] = bias_ih[RZ:H3]  # i_n bias only
    bias_row_hh = np.zeros((1, H3), np.float32)
    bias_row_hh[0, RZ:H3] = bias_hh[RZ:H3]  # h_n bias (inside r*(.))
    return {
        "wih1": np.ascontiguousarray(wihT[0:128]),
        "wih2": np.ascontiguousarray(np.concatenate([wihT[128:172], bias_row_ih])),
        "whh1": np.ascontiguousarray(whhT[0:128]),
        "whh2": np.ascontiguousarray(np.concatenate([whhT[128:172], bias_row_hh])),
    }


def _install_ntff_hook():
    """Register the axon NTFF profiling hook (antenv.axon_hooks is absent in
    this image; synthesize it from the injected libaxon_pjrt ABI)."""
    if "antenv.axon_hooks" in sys.modules:
        return
    so_path = "/opt/axon/libaxon_pjrt.so"
    try:
        lib = ctypes.CDLL(so_path)
    except OSError:
        return
    if not hasattr(lib, "axon_start_nrt_profile"):
        return
    lib.axon_start_nrt_profile.argtypes = [
        ctypes.POINTER(ctypes.c_int64),
        ctypes.c_size_t,
    ]
    lib.axon_start_nrt_profile.restype = ctypes.c_int64
    lib.axon_stop_nrt_profile.argtypes = [ctypes.c_char_p]
    lib.axon_stop_nrt_profile.restype = ctypes.c_int64

    @contextlib.contextmanager
    def _hook(output_dir, device_ids):
        import jax

        jax.devices()
        if device_ids:
            arr = (ctypes.c_int64 * len(device_ids))(*device_ids)
            rc = lib.axon_start_nrt_profile(arr, len(device_ids))
        else:
            rc = lib.axon_start_nrt_profile(None, 0)
        if rc != 0:
            raise RuntimeError(f"axon_start_nrt_profile rc={rc}")
        try:
            yield
        finally:
            n = lib.axon_stop_nrt_profile(str(output_dir).encode())
            if n < 0:
                raise RuntimeError(f"axon_stop_nrt_profile rc={n}")

    mod = types.ModuleType("antenv.axon_hooks")
    mod.get_axon_ntff_profile_hook = lambda: _hook
    sys.modules["antenv.axon_hooks"] = mod


_program_cache: dict = {}


def _get_program(r_shard, nt, g=G, nchunk=NCHUNK):
    key = (r_shard, nt, g, nchunk)
    if key not in _program_cache:
        _program_cache[key] = build_program(r_shard, nt, g, nchunk)
    return _program_cache[key]


def _run(inputs: dict, trace: bool = False, trace_cores=None):
    memory = np.asarray(inputs["memory"], np.float32)
    last_update = np.asarray(inputs["last_update"], np.float32)
    ids64 = np.asarray(inputs["unique_node_ids"]).astype(np.int64)
    msgs = np.asarray(inputs["unique_messages"], np.float32)
    ts = np.asarray(inputs["timestamps"], np.float32)

    # Capacity: tiles-of-128, rounded to a whole number of groups.
    counts = np.bincount(ids64 // R_SHARD, minlength=N_CORES)
    ng = max(1, math.ceil(counts.max() / (128 * G)))
    nt = ng * G
    mcap = nt * 128

    w_maps = make_weight_inputs(
        np.asarray(inputs["weight_ih"], np.float32),
        np.asarray(inputs["weight_hh"], np.float32),
        np.asarray(inputs["bias_ih"], np.float32),
        np.asarray(inputs["bias_hh"], np.float32),
    )
    in_maps = []
    for c in range(N_CORES):
        m = prepare_core_inputs(
            memory, last_update, ids64, msgs, ts, R_SHARD, mcap, c
        )
        m.update(w_maps)
        in_maps.append(m)

    nc = _get_program(R_SHARD, nt)

    if trace:
        _install_ntff_hook()
    res = run_bass_kernel_spmd(
        nc,
        in_maps,
        core_ids=list(range(N_CORES)),
        trace=trace,
        trace_cores=trace_cores,
    )

    mem_full = np.concatenate(
        [res.results[c]["mem_out"][:R_SHARD] for c in range(N_CORES)], axis=0
    )
    lu_full = np.concatenate(
        [res.results[c]["lu_out"][:R_SHARD] for c in range(N_CORES)]
    )
    return (mem_full[:, None, :], lu_full), res


def kernel(**inputs):
    out, _ = _run(inputs, trace=False)
    return out
